# revision 46
# baseline (speedup 1.0000x reference)
"""Trainium2 Bass kernel for the GAT+HSPA cascade (nn_GAT_HSPA_Cascade).

Takes FULL inputs (B=32), shards batch across 8 NeuronCores (4 per core,
weights replicated), runs one SPMD Bass/Tile program, gathers full output.

Per-batch pipeline on each core (b = one image):
  xf_t = Wq@xf+bq ; xf_g = relu(bn(Wg@xf+bg)) ; zf_t, zf_g likewise
  sim  = xf_t^T zf_t ; attn = softmax_m(sim)        [961 x 49]
  emb  = zf_g attn^T                                 [256 x 961]
  gat  = relu(bn(Wfi@[emb;xf_g]))  (computed in BOTH orientations on PE)
  e    = prelu(bn(conv3x3(gat)))                     [64 x 961]
  S    = e^T e                                       [961 x 961]
  P    = sparsemax_row(S) via top-16 (max8 over even/odd cols + bitonic
         merge + tau = max_j (cumsum_j - 1)/j), exact for support <= 15
  out  = gat @ P + gat
"""

import numpy as np

import concourse.bass as bass
import concourse.mybir as mybir
from concourse import bacc
from concourse.tile import TileContext
from concourse.masks import make_identity
from concourse.bass_utils import run_bass_kernel_spmd

F32 = mybir.dt.float32
F32R = mybir.dt.float32r
BF16 = mybir.dt.bfloat16
AF = mybir.ActivationFunctionType
ALU = mybir.AluOpType

# problem constants (hardcoded per contract)
B, C, CM = 32, 256, 64
HX, WX, HZ, WZ = 31, 31, 7, 7
NX, NZ = HX * WX, HZ * WZ          # 961, 49
NCORES = 8
BL = B // NCORES                   # 4 batches per core
EPS = 1e-5
PW = HX + 2                        # padded spatial width 33
EXP_SHIFT = -40.0                  # constant softmax shift (sim |max| ~ 60)

# flat splits of the 961-wide free dim, 512-aligned for PSUM banks.
# fp32r matmul dst needs EVEN element counts -> pad rhs tiles to 962 cols.
FSPLITS = [(0, 512), (512, 450)]
# bf16 final matmul has no even-N constraint
OSPLITS = [(0, 512), (512, 449)]
# conv row-splits: windows widened to 32 (even) -> psum offsets 0 / 512
CSPLITS = [(0, 16, 0), (16, 15, 512)]
PWW = 34  # padded gat buffer row width (31 + 1 left + 2 right)
# 961 rows -> 8 partition chunks
NCH = [(ci * 128, min(128, NX - ci * 128)) for ci in range(8)]

# matmul dtype per layer: 'f32' (exact, 4 cyc/col) or 'f32r' (1 cyc/col)
MM_DT = {
    't': 'f32r', 'g': 'f32r', 'sim': 'f32', 'emb': 'f32r',
    'fi': 'f32r', 'conv': 'f32r', 'S': 'f32r',
}


def _r(ap, kind):
    """Cast an fp32 AP for matmul input per layer config."""
    if kind == 'f32':
        return ap
    return ap.bitcast(F32R)


def build_bass():
    nc = bacc.Bacc(None, target_bir_lowering=False)

    # ---------------- DRAM I/O ----------------
    zf_h = nc.dram_tensor("zf", [BL, C, HZ, WZ], F32, kind="ExternalInput")
    xf_h = nc.dram_tensor("xf", [BL, C, HX, WX], F32, kind="ExternalInput")
    Wq_h = nc.dram_tensor("Wq", [C, C], F32, kind="ExternalInput")
    bq_h = nc.dram_tensor("bq", [C], F32, kind="ExternalInput")
    Ws_h = nc.dram_tensor("Ws", [C, C], F32, kind="ExternalInput")
    bs_h = nc.dram_tensor("bs", [C], F32, kind="ExternalInput")
    Wg_h = nc.dram_tensor("Wg", [C, C], F32, kind="ExternalInput")
    bg_h = nc.dram_tensor("bg", [C], F32, kind="ExternalInput")
    g_gamma_h = nc.dram_tensor("g_gamma", [C], F32, kind="ExternalInput")
    g_beta_h = nc.dram_tensor("g_beta", [C], F32, kind="ExternalInput")
    g_mean_h = nc.dram_tensor("g_mean", [C], F32, kind="ExternalInput")
    g_var_h = nc.dram_tensor("g_var", [C], F32, kind="ExternalInput")
    Wfi_h = nc.dram_tensor("Wfi", [C, 2 * C], F32, kind="ExternalInput")
    bfi_h = nc.dram_tensor("bfi", [C], F32, kind="ExternalInput")
    fi_gamma_h = nc.dram_tensor("fi_gamma", [C], F32, kind="ExternalInput")
    fi_beta_h = nc.dram_tensor("fi_beta", [C], F32, kind="ExternalInput")
    fi_mean_h = nc.dram_tensor("fi_mean", [C], F32, kind="ExternalInput")
    fi_var_h = nc.dram_tensor("fi_var", [C], F32, kind="ExternalInput")
    Wm_h = nc.dram_tensor("Wm", [CM, C, 3, 3], F32, kind="ExternalInput")
    bm_h = nc.dram_tensor("bm", [CM], F32, kind="ExternalInput")
    m_gamma_h = nc.dram_tensor("m_gamma", [CM], F32, kind="ExternalInput")
    m_beta_h = nc.dram_tensor("m_beta", [CM], F32, kind="ExternalInput")
    m_mean_h = nc.dram_tensor("m_mean", [CM], F32, kind="ExternalInput")
    m_var_h = nc.dram_tensor("m_var", [CM], F32, kind="ExternalInput")
    prelu_h = nc.dram_tensor("prelu_a", [1], F32, kind="ExternalInput")
    out_h = nc.dram_tensor("out", [BL, C, HX, WX], F32, kind="ExternalOutput")

    from contextlib import ExitStack
    with TileContext(nc) as tc, ExitStack() as ctx:
        wpool = ctx.enter_context(tc.tile_pool(name="weights", bufs=1))
        apool = ctx.enter_context(tc.tile_pool(name="acts", bufs=1))
        dbl = ctx.enter_context(tc.tile_pool(name="dbl", bufs=2))
        spool = ctx.enter_context(tc.tile_pool(name="smat", bufs=1))
        vpool = ctx.enter_context(tc.tile_pool(name="vsmall", bufs=2))
        pp = ctx.enter_context(tc.tile_pool(name="ps_big", bufs=2, space="PSUM"))
        pps = ctx.enter_context(tc.tile_pool(name="ps_small", bufs=4, space="PSUM"))

        ident = wpool.tile([128, 128], F32, tag="ident")
        make_identity(nc, ident)

        # register constant bias APs used by scalar.activation float biases
        for cval in (0.0, EPS, EXP_SHIFT):
            kt = wpool.tile([128, 1], F32, tag=f"konst_{cval}")
            nc.vector.memset(kt, cval)
            nc.const_aps.aps[(F32, cval)] = kt[:]

        # ---------------- load + transpose weights ----------------
        def load_wT(h, kin, kout, name):
            """DRAM [kout, kin] -> SBUF transposed [128, kin//128, kout]."""
            ko_ch, ki_ch = kout // 128, kin // 128
            raw = wpool.tile([128, ko_ch, kin], F32, tag=f"{name}_raw")
            nc.sync.dma_start(out=raw, in_=h[:].rearrange("(a p) i -> p a i", p=128))
            wt = wpool.tile([128, ki_ch, kout], F32R, tag=f"{name}T")
            for a in range(ko_ch):          # output-channel chunk
                for k in range(ki_ch):      # input-channel chunk
                    ps = pps.tile([128, 128], F32, tag="psmall")
                    nc.tensor.transpose(ps, raw[:, a, k * 128:(k + 1) * 128], ident)
                    nc.scalar.activation(wt[:, k, a * 128:(a + 1) * 128], ps, AF.Copy)
            return wt

        WqT = load_wT(Wq_h, C, C, "wq")
        WsT = load_wT(Ws_h, C, C, "ws")
        WgT = load_wT(Wg_h, C, C, "wg")

        # Wm [64, 256, 3, 3] -> WmT [128, 2, 9, 64]
        wm_raw = wpool.tile([CM, C, 9], F32, tag="wm_raw")
        nc.sync.dma_start(out=wm_raw, in_=Wm_h[:].rearrange("o i kh kw -> o i (kh kw)"))
        WmT = wpool.tile([128, 2, 9, CM], F32R, tag="wmT")
        for k in range(2):
            for t in range(9):
                ps = pps.tile([128, 128], F32, tag="psmall")
                nc.tensor.transpose(ps[:, :CM], wm_raw[:, k * 128:(k + 1) * 128, t],
                                    ident[:CM, :CM])
                nc.scalar.activation(WmT[:, k, t, :], ps[:, :CM], AF.Copy)

        # ---------------- per-channel vectors & BN folds ----------------
        def load_vec(h, n, name):
            ch = max(n // 128, 1)
            p = min(n, 128)
            t = wpool.tile([p, ch], F32, tag=f"v_{name}")
            nc.sync.dma_start(out=t, in_=h[:].rearrange("(a p) -> p a", p=p))
            return t

        bq_sb = load_vec(bq_h, C, "bq")
        bs_sb = load_vec(bs_h, C, "bs")
        bg_sb = load_vec(bg_h, C, "bg")
        g_gm = load_vec(g_gamma_h, C, "g_gm")
        g_bt = load_vec(g_beta_h, C, "g_bt")
        g_mn = load_vec(g_mean_h, C, "g_mn")
        g_vr = load_vec(g_var_h, C, "g_vr")
        bfi_sb = load_vec(bfi_h, C, "bfi")
        fi_gm = load_vec(fi_gamma_h, C, "fi_gm")
        fi_bt = load_vec(fi_beta_h, C, "fi_bt")
        fi_mn = load_vec(fi_mean_h, C, "fi_mn")
        fi_vr = load_vec(fi_var_h, C, "fi_vr")
        bm_sb = load_vec(bm_h, CM, "bm")
        m_gm = load_vec(m_gamma_h, CM, "m_gm")
        m_bt = load_vec(m_beta_h, CM, "m_bt")
        m_mn = load_vec(m_mean_h, CM, "m_mn")
        m_vr = load_vec(m_var_h, CM, "m_vr")
        prelu_sb = wpool.tile([CM, 1], F32, tag="prelu")
        nc.sync.dma_start(out=prelu_sb, in_=prelu_h[:].unsqueeze(0).to_broadcast([CM, 1]))

        def bn_fold(gm, bt, mn, vr, conv_b, p, ch, name):
            """Return (scale, bias_total): y = scale*x_conv + bias_total where
            x_conv = W@x (pre conv-bias): bias_total = s*conv_b + beta - mean*s."""
            s = wpool.tile([p, ch], F32, tag=f"bn_s_{name}")
            btot = wpool.tile([p, ch], F32, tag=f"bn_b_{name}")
            tmp = vpool.tile([p, ch], F32, tag="bn_tmp")
            nc.scalar.activation(tmp, vr, AF.Sqrt, bias=EPS)
            nc.vector.reciprocal(tmp, tmp)
            nc.vector.tensor_mul(s, gm, tmp)
            nc.vector.tensor_mul(tmp, mn, s)
            nc.vector.tensor_sub(btot, bt, tmp)          # beta - mean*s
            if conv_b is not None:
                tmp2 = vpool.tile([p, ch], F32, tag="bn_tmp2")
                nc.vector.tensor_mul(tmp2, conv_b, s)
                nc.vector.tensor_add(btot, btot, tmp2)   # + s*conv_b
            return s, btot

        g_s, g_b = bn_fold(g_gm, g_bt, g_mn, g_vr, bg_sb, 128, 2, "g")
        neg_a = wpool.tile([CM, 1], F32, tag="neg_a")
        nc.vector.tensor_scalar(neg_a, prelu_sb, -1.0, None, ALU.mult)
        fi_s, fi_b = bn_fold(fi_gm, fi_bt, fi_mn, fi_vr, bfi_sb, 128, 2, "fi")
        m_s, m_b = bn_fold(m_gm, m_bt, m_mn, m_vr, bm_sb, CM, 1, "m")
        m_s_neg = wpool.tile([CM, 1], F32, tag="m_s_neg")
        m_b_neg = wpool.tile([CM, 1], F32, tag="m_b_neg")
        nc.vector.tensor_scalar(m_s_neg, m_s, -1.0, None, ALU.mult)
        nc.vector.tensor_scalar(m_b_neg, m_b, -1.0, None, ALU.mult)

        # Fold fi scale into WfiT weights so gat^T (n-major) needs no
        # per-channel scale: WfiT_scaled[i, o] = Wfi[o, i] * fi_s[o].
        wfi_raw = wpool.tile([128, 2, 2 * C], F32, tag="wfi_raw")
        nc.sync.dma_start(out=wfi_raw, in_=Wfi_h[:].rearrange("(a p) i -> p a i", p=128))
        for a in range(2):
            nc.scalar.activation(wfi_raw[:, a, :], wfi_raw[:, a, :], AF.Identity,
                                 bias=0.0, scale=fi_s[:, a:a + 1])
        WfiT = wpool.tile([128, 4, C], F32R, tag="wfiT")
        for a in range(2):
            for k in range(4):
                ps = pps.tile([128, 128], F32, tag="psmall")
                nc.tensor.transpose(ps, wfi_raw[:, a, k * 128:(k + 1) * 128], ident)
                nc.scalar.activation(WfiT[:, k, a * 128:(a + 1) * 128], ps, AF.Copy)
        # gat bias row (for gat^T orientation, added via rank-1 matmul):
        # fi_btot[o] = fi_s*bfi + (beta - mean*s) ... = fi_b already includes
        # conv bias term. Need it as a [1, 256] row and ones [1, 128] col.
        fi_b_row = wpool.tile([1, C], F32R, tag="fi_b_row")
        for a in range(2):
            ps = pps.tile([128, 128], F32, tag="psmall")
            nc.tensor.transpose(ps[:1, :], fi_b[:, a:a + 1], ident)
            nc.scalar.activation(fi_b_row[:, a * 128:(a + 1) * 128], ps[:1, :], AF.Copy)
        ones_row = wpool.tile([1, 128], F32R, tag="ones_row")
        nc.vector.memset(ones_row.bitcast(F32), 1.0)
        ones49 = wpool.tile([NZ, 1], F32R, tag="ones49")
        nc.vector.memset(ones49.bitcast(F32), 1.0)
        ident_bf = wpool.tile([128, 128], BF16, tag="ident_bf")
        nc.vector.tensor_copy(ident_bf, ident)

        # 1/j constants for the sparsemax threshold, j = 1..16
        rj = wpool.tile([128, 1, 16], F32, tag="rj")
        for j in range(16):
            nc.vector.memset(rj[:, :, j:j + 1], 1.0 / (j + 1))

        # ---------------- software-pipelined per-batch stages ----------------
        # Emission order interleaves batch b's front matmuls with batch
        # b-1's sparsemax (DVE/ACT) + out-matmul so the PE never idles
        # long enough for HAM to re-throttle.
        st = [dict() for _ in range(BL)]

        def emit_load(b):
            s = st[b]
            s['xf_sb'] = dbl.tile([128, 2, NX], F32, tag="xf", name="xf_sb")
            nc.sync.dma_start(
                out=s['xf_sb'],
                in_=xf_h[b].rearrange("(a p) h w -> p a (h w)", p=128))
            s['zf_sb'] = dbl.tile([128, 2, NZ], F32, tag="zf", name="zf_sb")
            nc.sync.dma_start(
                out=s['zf_sb'],
                in_=zf_h[b].rearrange("(a p) h w -> p a (h w)", p=128))
            # round inputs to f32r right away: these gate every front matmul,
            # so they must precede the previous batch's max8s in the DVE queue
            xf_r = apool.tile([128, 2, NX + 1], F32R, tag="xf_r", name="xf_r")
            nc.vector.tensor_copy(xf_r[:, :, :NX], s['xf_sb'])
            nc.vector.memset(xf_r[:, :, NX:].bitcast(F32), 0.0)
            s['xf_r'] = xf_r
            zf_r = apool.tile([128, 2, NZ + 1], F32R, tag="zf_r", name="zf_r")
            nc.vector.tensor_copy(zf_r[:, :, :NZ], s['zf_sb'])
            nc.vector.memset(zf_r[:, :, NZ:].bitcast(F32), 0.0)
            s['zf_r'] = zf_r

        def emit_front_a(b):
            s = st[b]
            xf_r, zf_r = s['xf_r'], s['zf_r']

            # -- t path feeds sim: do it first --
            xf_t = apool.tile([128, 2, NX + 1], F32, tag="xf_t")
            nc.vector.memset(xf_t[:, :, NX:], 0.0)
            for a in range(2):
                ps_t = pp.tile([128, 1024], F32, tag="mm961")
                for f0, fw in FSPLITS:
                    sl = slice(f0, f0 + fw)
                    for k in range(2):
                        nc.tensor.matmul(
                            ps_t[:, sl], WqT[:, k, a * 128:(a + 1) * 128],
                            xf_r[:, k, sl],
                            start=(k == 0), stop=(k == 1))
                nc.scalar.activation(xf_t[:, a, :NX], ps_t[:, :NX], AF.Identity,
                                     bias=bq_sb[:, a:a + 1])
            zf_t = apool.tile([128, 2, NZ], F32, tag="zf_t")
            for a in range(2):
                ps_t = pps.tile([128, NZ + 1], F32, tag="psmall")
                for k in range(2):
                    nc.tensor.matmul(ps_t, WsT[:, k, a * 128:(a + 1) * 128],
                                     zf_r[:, k, :], start=(k == 0), stop=(k == 1))
                nc.scalar.activation(zf_t[:, a, :], ps_t[:, :NZ], AF.Identity,
                                     bias=bs_sb[:, a:a + 1])

            # simT = zf_t^T xf_t [49, 961] (fp32)
            ps_sT = pp.tile([NZ, 1024], F32, tag="mm961")
            for f0, fw in FSPLITS:
                sl = slice(f0, f0 + fw)
                for k in range(2):
                    nc.tensor.matmul(ps_sT[:, sl], zf_t[:, k, :],
                                     xf_t[:, k, sl], start=(k == 0), stop=(k == 1))
            expT = apool.tile([NZ, NX + 1], F32R, tag="expT")
            nc.vector.memset(expT[:, NX:].bitcast(F32), 0.0)
            nc.scalar.activation(expT[:, :NX], ps_sT[:, :NX], AF.Exp,
                                 bias=EXP_SHIFT)
            # unnormalized emb = zf_g @ expT can start immediately; the
            # softmax denominator commutes through this linear map and is
            # applied on the PSUM->SBUF copy below.
            ps_den = pp.tile([1, 1024], F32, tag="mm961")
            for f0, fw in FSPLITS:
                sl = slice(f0, f0 + fw)
                nc.tensor.matmul(ps_den[:, sl], ones49, expT[:, sl],
                                 start=True, stop=True)
            # rden = 1/den via exp(-ln(den)) on ACT (rden ~O(1): f32r-safe)
            lnden = apool.tile([1, NX], F32, tag="lnden")
            nc.scalar.activation(lnden, ps_den[:, :NX], AF.Ln)
            rden = apool.tile([1, NX + 1], F32R, tag="rden_row")
            nc.vector.memset(rden.bitcast(F32), 0.0)
            nc.scalar.activation(rden[:, :NX], lnden, AF.Exp, scale=-1.0)

            # -- g path fills the PE while ln/exp cook --
            xf_g = apool.tile([128, 2, NX + 1], F32R, tag="xf_g")
            nc.vector.memset(xf_g[:, :, NX:].bitcast(F32), 0.0)
            for a in range(2):
                ps_g = pp.tile([128, 1024], F32, tag="mm961")
                for f0, fw in FSPLITS:
                    sl = slice(f0, f0 + fw)
                    for k in range(2):
                        nc.tensor.matmul(
                            ps_g[:, sl], WgT[:, k, a * 128:(a + 1) * 128],
                            xf_r[:, k, sl],
                            start=(k == 0), stop=(k == 1))
                nc.scalar.activation(xf_g[:, a, :NX], ps_g[:, :NX], AF.Relu,
                                     bias=g_b[:, a:a + 1], scale=g_s[:, a:a + 1])

            # broadcast rden across partitions via rank-1 f32r matmul
            ps_bc = pp.tile([128, 1024], F32, tag="mm961")
            for f0, fw in FSPLITS:
                sl = slice(f0, f0 + fw)
                nc.tensor.matmul(ps_bc[:, sl], ones_row, rden[:, sl],
                                 start=True, stop=True)

            zf_g = apool.tile([128, 2, NZ], F32, tag="zf_g")
            for a in range(2):
                ps_g = pps.tile([128, NZ + 1], F32, tag="psmall")
                for k in range(2):
                    nc.tensor.matmul(ps_g, WgT[:, k, a * 128:(a + 1) * 128],
                                     zf_r[:, k, :], start=(k == 0), stop=(k == 1))
                nc.scalar.activation(zf_g[:, a, :], ps_g[:, :NZ], AF.Relu,
                                     bias=g_b[:, a:a + 1], scale=g_s[:, a:a + 1])
            # zf_g^T [49, 256] for emb matmul
            zf_gT = apool.tile([NZ, C], F32R, tag="zf_gT")
            for k in range(2):
                ps = pps.tile([128, 128], F32, tag="psmall")
                nc.tensor.transpose(ps[:NZ, :], zf_g[:, k, :], ident)
                nc.scalar.activation(zf_gT[:, k * 128:(k + 1) * 128], ps[:NZ, :],
                                     AF.Copy)


            rdbc = apool.tile([128, NX], F32, tag="rdbc")
            nc.scalar.activation(rdbc, ps_bc[:, :NX], AF.Copy)

            # emb = (zf_g @ expT) * rden  [256, 961]
            emb = apool.tile([128, 2, NX + 1], F32R, tag="emb")
            nc.vector.memset(emb[:, :, NX:].bitcast(F32), 0.0)
            for a in range(2):
                ps = pp.tile([128, 1024], F32, tag="mm961")
                for f0, fw in FSPLITS:
                    sl = slice(f0, f0 + fw)
                    nc.tensor.matmul(ps[:, sl],
                                     zf_gT[:, a * 128:(a + 1) * 128],
                                     expT[:, sl], start=True, stop=True)
                nc.vector.tensor_mul(emb[:, a, :NX], ps[:, :NX], rdbc)

            # gat (c-major, padded) -- double-buffered across batches
            gat_pad = dbl.tile([128, 2, PW, PWW], F32R, tag="gat_pad")
            s['gat_pad'] = gat_pad
            for a in range(2):
                nc.vector.memset(gat_pad[:, a, 0, :].bitcast(F32), 0.0)
                nc.vector.memset(gat_pad[:, a, PW - 1, :].bitcast(F32), 0.0)
                nc.vector.memset(gat_pad[:, a, 1:PW - 1, 0:1].bitcast(F32), 0.0)
                nc.vector.memset(gat_pad[:, a, 1:PW - 1, 1 + WX:PWW].bitcast(F32),
                                 0.0)
                ps = pp.tile([128, 1024], F32, tag="mm961")
                for f0, fw in FSPLITS:
                    sl = slice(f0, f0 + fw)
                    for k in range(4):
                        rhs = emb[:, k, sl] if k < 2 else xf_g[:, k - 2, sl]
                        nc.tensor.matmul(ps[:, sl],
                                         WfiT[:, k, a * 128:(a + 1) * 128],
                                         rhs, start=(k == 0), stop=(k == 3))
                nc.scalar.activation(
                    gat_pad[:, a, 1:1 + HX, 1:1 + WX],
                    ps[:, :NX].rearrange("p (h w) -> p h w", w=WX),
                    AF.Relu, bias=fi_b[:, a:a + 1])

            # contiguous bf16 copy of gat for transposing (split DVE/ACT)
            gat_sb = apool.tile([128, 2, NX], BF16, tag="gat_sb")
            nc.vector.tensor_copy(
                gat_sb[:, 0, :].rearrange("p (h w) -> p h w", w=WX),
                gat_pad[:, 0, 1:1 + HX, 1:1 + WX].bitcast(F32))
            nc.vector.tensor_copy(
                gat_sb[:, 1, :].rearrange("p (h w) -> p h w", w=WX),
                gat_pad[:, 1, 1:1 + HX, 1:1 + WX].bitcast(F32))
            # gat^T (n-major, bf16) via PE transposes (bf16: FWL + 1 cyc/col)
            gatT = dbl.tile([128, 8, C], BF16, tag="gatT")
            s['gatT'] = gatT
            for ci, (n0, cs) in enumerate(NCH):
                for a in range(2):
                    ps = pps.tile([128, 128], BF16, tag="psmall")
                    nc.tensor.transpose(ps[:cs, :], gat_sb[:, a, n0:n0 + cs],
                                        ident_bf)
                    nc.vector.tensor_copy(gatT[:cs, ci, a * 128:(a + 1) * 128],
                                          ps[:cs, :])

        def emit_conv(b):
            s = st[b]
            gat_pad = s['gat_pad']
            # e = prelu(bn(conv3x3(gat))) -> ef [64, 961(+1 pad)]
            # full 128 partitions: rows 64..127 get a DMA replica for S packing
            ef = apool.tile([2 * CM, NX + 1], F32R, tag="ef")
            nc.vector.memset(ef[:, NX:].bitcast(F32), 0.0)
            ps_e = pp.tile([CM, 1024], F32, tag="mm961")
            for r0, nr, p0 in CSPLITS:
                first = True
                for t in range(9):
                    dy, dx = t // 3, t % 3
                    for k in range(2):
                        nc.tensor.matmul(
                            ps_e[:, p0:p0 + nr * 32], WmT[:, k, t, :],
                            gat_pad[:, k, dy + r0:dy + r0 + nr, dx:dx + 32],
                            start=first, stop=(t == 8 and k == 1))
                        first = False
            # prelu(y) = relu(y) - a*relu(-y), y = m_s*conv + m_b
            efn = apool.tile([CM, NX], F32, tag="efn")
            for r0, nr, p0 in CSPLITS:
                pv = ps_e[:, p0:p0 + nr * 32].rearrange(
                    "p (r w) -> p r w", w=32)[:, :, :WX]
                ov = slice(r0 * WX, (r0 + nr) * WX)
                nc.scalar.activation(
                    ef[:CM, ov].rearrange("p (r w) -> p r w", w=WX), pv,
                    AF.Relu, bias=m_b, scale=m_s)
                nc.scalar.activation(
                    efn[:, ov].rearrange("p (r w) -> p r w", w=WX), pv,
                    AF.Relu, bias=m_b_neg, scale=m_s_neg)
                nc.vector.scalar_tensor_tensor(
                    ef[:CM, ov], efn[:, ov], neg_a[:, 0:1],
                    ef[:CM, ov].bitcast(F32), op0=ALU.mult, op1=ALU.add)
                # replicate this split to partitions 64..127 immediately
                nc.sync.dma_start(out=ef[CM:2 * CM, ov], in_=ef[:CM, ov])
            nc.sync.dma_start(out=ef[CM:2 * CM, NX:], in_=ef[:CM, NX:])

            s['ef'] = ef

        def emit_S(b, early_max8=False):
            s = st[b]
            ef = s['ef']
            # S = ef^T ef  [961, 961]; chunk pairs run concurrently in PE
            # row-groups 0-63 / 64-127 (K=64 each)
            S_sb = spool.tile([128, 8, NX], F32, tag="S")
            s['S_sb'] = S_sb
            if early_max8:
                v16 = vpool.tile([128, 8, 16], F32, tag="v16", name="v16")
                nc.vector.memset(v16, 0.0)
                s['v16'] = v16
            for p in range(4):
                c0, c1 = 2 * p, 2 * p + 1
                n00, cs0 = NCH[c0]
                n01, cs1 = NCH[c1]
                psA = pp.tile([128, 1024], F32, tag="mm961")
                psB = pp.tile([128, 1024], F32, tag="mm961")
                for f0, fw in FSPLITS:
                    sl = slice(f0, f0 + fw)
                    nc.tensor.matmul(psA[:cs0, sl], ef[:CM, n00:n00 + cs0],
                                     ef[:CM, sl], start=True, stop=True)
                    nc.tensor.matmul(psB[:cs1, sl],
                                     ef[CM:2 * CM, n01:n01 + cs1],
                                     ef[CM:2 * CM, sl], start=True, stop=True)
                nc.vector.tensor_copy(S_sb[:cs0, c0, :], psA[:cs0, :NX])
                nc.scalar.activation(S_sb[:cs1, c1, :], psB[:cs1, :NX], AF.Copy)
                if early_max8:
                    for cc, css in ((c0, cs0), (c1, cs1)):
                        nc.vector.max(out=s['v16'][:css, cc, 0:8],
                                      in_=S_sb[:css, cc, 0:NX:2])
                        nc.vector.max(out=s['v16'][:css, cc, 8:16][:, ::-1],
                                      in_=S_sb[:css, cc, 1:NX:2])

        def emit_topk(b):
            s = st[b]
            S_sb = s['S_sb']
            if 'v16' in s:
                v16 = s['v16']
            else:
                v16 = vpool.tile([128, 8, 16], F32, tag="v16")
                nc.vector.memset(v16, 0.0)
                for ci, (n0, cs) in enumerate(NCH):
                    nc.vector.max(out=v16[:cs, ci, 0:8],
                                  in_=S_sb[:cs, ci, 0:NX:2])
                    nc.vector.max(out=v16[:cs, ci, 8:16][:, ::-1],
                                  in_=S_sb[:cs, ci, 1:NX:2])
            mA = vpool.tile([128, 8, 16], F32, tag="mA")
            mB = vpool.tile([128, 8, 16], F32, tag="mB")
            # bitonic merge (descending) of [desc8 ; asc8]
            nc.vector.tensor_tensor(mA[:, :, 0:8], v16[:, :, 0:8],
                                    v16[:, :, 8:16], ALU.max)
            nc.vector.tensor_tensor(mA[:, :, 8:16], v16[:, :, 0:8],
                                    v16[:, :, 8:16], ALU.min)
            for (mm0, mm1, g, x) in ((mA, mB, 2, 8), (mB, mA, 4, 4),
                                     (mA, mB, 8, 2)):
                s4 = mm0.rearrange("p c (g x) -> p c g x", g=g)
                d4 = mm1.rearrange("p c (g x) -> p c g x", g=g)
                h = x // 2
                nc.vector.tensor_tensor(d4[:, :, :, 0:h], s4[:, :, :, 0:h],
                                        s4[:, :, :, h:x], ALU.max)
                nc.vector.tensor_tensor(d4[:, :, :, h:x], s4[:, :, :, 0:h],
                                        s4[:, :, :, h:x], ALU.min)
            # mB sorted descending; cumsum (Hillis-Steele, ping-pong)
            for (mm0, mm1, sh) in ((mB, mA, 1), (mA, v16, 2), (v16, mA, 4),
                                   (mA, v16, 8)):
                nc.vector.tensor_tensor(mm1[:, :, sh:16], mm0[:, :, sh:16],
                                        mm0[:, :, 0:16 - sh], ALU.add)
                nc.vector.tensor_copy(mm1[:, :, 0:sh], mm0[:, :, 0:sh])
            # t_j = (cumsum_j - 1) / j ; tau = max_j t_j
            nc.vector.scalar_tensor_tensor(mB, v16, 1.0,
                                           rj.to_broadcast([128, 8, 16]),
                                           op0=ALU.subtract, op1=ALU.mult)
            tau = vpool.tile([128, 8], F32, tag="tau")
            nc.vector.tensor_reduce(tau, mB, mybir.AxisListType.X, ALU.max)
            ntau = vpool.tile([128, 8], F32, tag="ntau")
            nc.vector.tensor_scalar(ntau, tau, -1.0, None, ALU.mult)
            s['tau'] = tau
            s['ntau'] = ntau

        def emit_P(b):
            # P = relu(S - tau) in bf16, split DVE/ACT; emitted late so the
            # ACT chunks don't head-of-line-block the next batch's copies
            s = st[b]
            S_sb, tau, ntau = s['S_sb'], s['tau'], s['ntau']
            P_sb = spool.tile([128, 8, NX], BF16, tag="P")
            s['P_sb'] = P_sb
            for ci, (n0, cs) in enumerate(NCH):
                if ci % 2 == 0:
                    nc.vector.tensor_scalar(P_sb[:cs, ci, :], S_sb[:cs, ci, :],
                                            tau[:cs, ci:ci + 1], 0.0,
                                            ALU.subtract, ALU.max)
                else:
                    nc.scalar.activation(P_sb[:cs, ci, :], S_sb[:cs, ci, :],
                                         AF.Relu, bias=ntau[:cs, ci:ci + 1])

        def emit_out(b):
            s = st[b]
            gat_pad, gatT, P_sb = s['gat_pad'], s['gatT'], s['P_sb']
            out_sb = dbl.tile([128, 2, NX], F32, tag="out_sb")
            for a in range(2):
                ps = pp.tile([128, 1024], F32, tag="mm961")
                for f0, fw in OSPLITS:
                    sl = slice(f0, f0 + fw)
                    for ci, (n0, cs) in enumerate(NCH):
                        nc.tensor.matmul(ps[:, sl],
                                         gatT[:cs, ci, a * 128:(a + 1) * 128],
                                         P_sb[:cs, ci, sl],
                                         start=(ci == 0), stop=(ci == 7))
                nc.vector.tensor_add(
                    out_sb[:, a, :].rearrange("p (h w) -> p h w", w=WX),
                    ps[:, :NX].rearrange("p (h w) -> p h w", w=WX),
                    gat_pad[:, a, 1:1 + HX, 1:1 + WX].bitcast(F32))
            nc.sync.dma_start(
                out=out_h[b].rearrange("(a p) h w -> p a (h w)", p=128),
                in_=out_sb)

        emit_load(0)
        emit_front_a(0)
        emit_conv(0)
        emit_S(0)
        for b in range(1, BL):
            emit_load(b)
            emit_topk(b - 1)
            emit_front_a(b)
            emit_conv(b)
            emit_P(b - 1)
            emit_out(b - 1)
            emit_S(b, early_max8=(b == BL - 1))
        emit_topk(BL - 1)
        emit_P(BL - 1)
        emit_out(BL - 1)

    nc.compile()
    return nc


_CACHED = None


def _get_nc():
    global _CACHED
    if _CACHED is None:
        _CACHED = build_bass()
    return _CACHED


def kernel(**inputs):
    nc = _get_nc()
    full = {k: np.ascontiguousarray(np.asarray(v, dtype=np.float32))
            for k, v in inputs.items()}
    full['prelu_a'] = full['prelu_a'].reshape(1)
    in_maps = []
    for c in range(NCORES):
        m = dict(full)
        m['zf'] = full['zf'][c * BL:(c + 1) * BL]
        m['xf'] = full['xf'][c * BL:(c + 1) * BL]
        in_maps.append(m)
    res = run_bass_kernel_spmd(nc, in_maps, core_ids=list(range(NCORES)))
    out = np.concatenate([r['out'] for r in res.results], axis=0)
    return out.astype(np.float32)


if __name__ == "__main__":
    # smoke-build
    nc = build_bass()
    print("built ok:",
          sum(len(b.instructions) for f in nc.m.functions for b in f.blocks),
          "instructions")


# revision 48
# speedup vs baseline: 1.0352x; 1.0352x over previous
"""Trainium2 Bass kernel for the GAT+HSPA cascade (nn_GAT_HSPA_Cascade).

Takes FULL inputs (B=32), shards batch across 8 NeuronCores (4 per core,
weights replicated), runs one SPMD Bass/Tile program, gathers full output.

Per-batch pipeline on each core (b = one image):
  xf_t = Wq@xf+bq ; xf_g = relu(bn(Wg@xf+bg)) ; zf_t, zf_g likewise
  sim  = xf_t^T zf_t ; attn = softmax_m(sim)        [961 x 49]
  emb  = zf_g attn^T                                 [256 x 961]
  gat  = relu(bn(Wfi@[emb;xf_g]))  (computed in BOTH orientations on PE)
  e    = prelu(bn(conv3x3(gat)))                     [64 x 961]
  S    = e^T e                                       [961 x 961]
  P    = sparsemax_row(S) via top-16 (max8 over even/odd cols + bitonic
         merge + tau = max_j (cumsum_j - 1)/j), exact for support <= 15
  out  = gat @ P + gat
"""

import numpy as np

import concourse.bass as bass
import concourse.mybir as mybir
from concourse import bacc
from concourse.tile import TileContext
from concourse.masks import make_identity
from concourse.bass_utils import run_bass_kernel_spmd

F32 = mybir.dt.float32
F32R = mybir.dt.float32r
BF16 = mybir.dt.bfloat16
AF = mybir.ActivationFunctionType
ALU = mybir.AluOpType

# problem constants (hardcoded per contract)
B, C, CM = 32, 256, 64
HX, WX, HZ, WZ = 31, 31, 7, 7
NX, NZ = HX * WX, HZ * WZ          # 961, 49
NCORES = 8
BL = B // NCORES                   # 4 batches per core
EPS = 1e-5
PW = HX + 2                        # padded spatial width 33
EXP_SHIFT = -40.0                  # constant softmax shift (sim |max| ~ 60)

# flat splits of the 961-wide free dim, 512-aligned for PSUM banks.
# fp32r matmul dst needs EVEN element counts -> pad rhs tiles to 962 cols.
FSPLITS = [(0, 512), (512, 450)]
# bf16 final matmul has no even-N constraint
OSPLITS = [(0, 512), (512, 449)]
# conv row-splits: windows widened to 32 (even) -> psum offsets 0 / 512
CSPLITS = [(0, 16, 0), (16, 15, 512)]
PWW = 34  # padded gat buffer row width (31 + 1 left + 2 right)
# 961 rows -> 8 partition chunks
NCH = [(ci * 128, min(128, NX - ci * 128)) for ci in range(8)]

# matmul dtype per layer: 'f32' (exact, 4 cyc/col) or 'f32r' (1 cyc/col)
MM_DT = {
    't': 'f32r', 'g': 'f32r', 'sim': 'f32', 'emb': 'f32r',
    'fi': 'f32r', 'conv': 'f32r', 'S': 'f32r',
}


def _r(ap, kind):
    """Cast an fp32 AP for matmul input per layer config."""
    if kind == 'f32':
        return ap
    return ap.bitcast(F32R)


def build_bass():
    nc = bacc.Bacc(None, target_bir_lowering=False)

    # ---------------- DRAM I/O ----------------
    zf_h = nc.dram_tensor("zf", [BL, C, HZ, WZ], F32, kind="ExternalInput")
    xf_h = nc.dram_tensor("xf", [BL, C, HX, WX], F32, kind="ExternalInput")
    Wq_h = nc.dram_tensor("Wq", [C, C], F32, kind="ExternalInput")
    bq_h = nc.dram_tensor("bq", [C], F32, kind="ExternalInput")
    Ws_h = nc.dram_tensor("Ws", [C, C], F32, kind="ExternalInput")
    bs_h = nc.dram_tensor("bs", [C], F32, kind="ExternalInput")
    Wg_h = nc.dram_tensor("Wg", [C, C], F32, kind="ExternalInput")
    bg_h = nc.dram_tensor("bg", [C], F32, kind="ExternalInput")
    g_gamma_h = nc.dram_tensor("g_gamma", [C], F32, kind="ExternalInput")
    g_beta_h = nc.dram_tensor("g_beta", [C], F32, kind="ExternalInput")
    g_mean_h = nc.dram_tensor("g_mean", [C], F32, kind="ExternalInput")
    g_var_h = nc.dram_tensor("g_var", [C], F32, kind="ExternalInput")
    Wfi_h = nc.dram_tensor("Wfi", [C, 2 * C], F32, kind="ExternalInput")
    bfi_h = nc.dram_tensor("bfi", [C], F32, kind="ExternalInput")
    fi_gamma_h = nc.dram_tensor("fi_gamma", [C], F32, kind="ExternalInput")
    fi_beta_h = nc.dram_tensor("fi_beta", [C], F32, kind="ExternalInput")
    fi_mean_h = nc.dram_tensor("fi_mean", [C], F32, kind="ExternalInput")
    fi_var_h = nc.dram_tensor("fi_var", [C], F32, kind="ExternalInput")
    Wm_h = nc.dram_tensor("Wm", [CM, C, 3, 3], F32, kind="ExternalInput")
    bm_h = nc.dram_tensor("bm", [CM], F32, kind="ExternalInput")
    m_gamma_h = nc.dram_tensor("m_gamma", [CM], F32, kind="ExternalInput")
    m_beta_h = nc.dram_tensor("m_beta", [CM], F32, kind="ExternalInput")
    m_mean_h = nc.dram_tensor("m_mean", [CM], F32, kind="ExternalInput")
    m_var_h = nc.dram_tensor("m_var", [CM], F32, kind="ExternalInput")
    prelu_h = nc.dram_tensor("prelu_a", [1], F32, kind="ExternalInput")
    out_h = nc.dram_tensor("out", [BL, C, HX, WX], F32, kind="ExternalOutput")

    from contextlib import ExitStack
    with TileContext(nc) as tc, ExitStack() as ctx:
        wpool = ctx.enter_context(tc.tile_pool(name="weights", bufs=1))
        apool = ctx.enter_context(tc.tile_pool(name="acts", bufs=1))
        dbl = ctx.enter_context(tc.tile_pool(name="dbl", bufs=2))
        spool = ctx.enter_context(tc.tile_pool(name="smat", bufs=1))
        vpool = ctx.enter_context(tc.tile_pool(name="vsmall", bufs=2))
        pp = ctx.enter_context(tc.tile_pool(name="ps_big", bufs=3, space="PSUM"))
        pps = ctx.enter_context(tc.tile_pool(name="ps_small", bufs=2, space="PSUM"))

        ident = wpool.tile([128, 128], F32, tag="ident")
        make_identity(nc, ident)

        # register constant bias APs used by scalar.activation float biases
        for cval in (0.0, EPS, EXP_SHIFT):
            kt = wpool.tile([128, 1], F32, tag=f"konst_{cval}")
            nc.vector.memset(kt, cval)
            nc.const_aps.aps[(F32, cval)] = kt[:]

        # ---------------- load + transpose weights ----------------
        def load_wT(h, kin, kout, name):
            """DRAM [kout, kin] -> SBUF transposed [128, kin//128, kout]."""
            ko_ch, ki_ch = kout // 128, kin // 128
            raw = wpool.tile([128, ko_ch, kin], F32, tag=f"{name}_raw")
            nc.sync.dma_start(out=raw, in_=h[:].rearrange("(a p) i -> p a i", p=128))
            wt = wpool.tile([128, ki_ch, kout], F32R, tag=f"{name}T")
            for a in range(ko_ch):          # output-channel chunk
                for k in range(ki_ch):      # input-channel chunk
                    ps = pps.tile([128, 128], F32, tag="psmall")
                    nc.tensor.transpose(ps, raw[:, a, k * 128:(k + 1) * 128], ident)
                    nc.scalar.activation(wt[:, k, a * 128:(a + 1) * 128], ps, AF.Copy)
            return wt

        WqT = load_wT(Wq_h, C, C, "wq")
        WsT = load_wT(Ws_h, C, C, "ws")
        WgT = load_wT(Wg_h, C, C, "wg")

        # Wm [64, 256, 3, 3] -> WmT [128, 2, 9, 64]
        wm_raw = wpool.tile([CM, C, 9], F32, tag="wm_raw")
        nc.sync.dma_start(out=wm_raw, in_=Wm_h[:].rearrange("o i kh kw -> o i (kh kw)"))
        WmT = wpool.tile([128, 2, 9, CM], F32R, tag="wmT")
        for k in range(2):
            for t in range(9):
                ps = pps.tile([128, 128], F32, tag="psmall")
                nc.tensor.transpose(ps[:, :CM], wm_raw[:, k * 128:(k + 1) * 128, t],
                                    ident[:CM, :CM])
                nc.scalar.activation(WmT[:, k, t, :], ps[:, :CM], AF.Copy)

        # ---------------- per-channel vectors & BN folds ----------------
        def load_vec(h, n, name):
            ch = max(n // 128, 1)
            p = min(n, 128)
            t = wpool.tile([p, ch], F32, tag=f"v_{name}")
            nc.sync.dma_start(out=t, in_=h[:].rearrange("(a p) -> p a", p=p))
            return t

        bq_sb = load_vec(bq_h, C, "bq")
        bs_sb = load_vec(bs_h, C, "bs")
        bg_sb = load_vec(bg_h, C, "bg")
        g_gm = load_vec(g_gamma_h, C, "g_gm")
        g_bt = load_vec(g_beta_h, C, "g_bt")
        g_mn = load_vec(g_mean_h, C, "g_mn")
        g_vr = load_vec(g_var_h, C, "g_vr")
        bfi_sb = load_vec(bfi_h, C, "bfi")
        fi_gm = load_vec(fi_gamma_h, C, "fi_gm")
        fi_bt = load_vec(fi_beta_h, C, "fi_bt")
        fi_mn = load_vec(fi_mean_h, C, "fi_mn")
        fi_vr = load_vec(fi_var_h, C, "fi_vr")
        bm_sb = load_vec(bm_h, CM, "bm")
        m_gm = load_vec(m_gamma_h, CM, "m_gm")
        m_bt = load_vec(m_beta_h, CM, "m_bt")
        m_mn = load_vec(m_mean_h, CM, "m_mn")
        m_vr = load_vec(m_var_h, CM, "m_vr")
        prelu_sb = wpool.tile([CM, 1], F32, tag="prelu")
        nc.sync.dma_start(out=prelu_sb, in_=prelu_h[:].unsqueeze(0).to_broadcast([CM, 1]))

        def bn_fold(gm, bt, mn, vr, conv_b, p, ch, name):
            """Return (scale, bias_total): y = scale*x_conv + bias_total where
            x_conv = W@x (pre conv-bias): bias_total = s*conv_b + beta - mean*s."""
            s = wpool.tile([p, ch], F32, tag=f"bn_s_{name}")
            btot = wpool.tile([p, ch], F32, tag=f"bn_b_{name}")
            tmp = vpool.tile([p, ch], F32, tag="bn_tmp")
            nc.scalar.activation(tmp, vr, AF.Sqrt, bias=EPS)
            nc.vector.reciprocal(tmp, tmp)
            nc.vector.tensor_mul(s, gm, tmp)
            nc.vector.tensor_mul(tmp, mn, s)
            nc.vector.tensor_sub(btot, bt, tmp)          # beta - mean*s
            if conv_b is not None:
                tmp2 = vpool.tile([p, ch], F32, tag="bn_tmp2")
                nc.vector.tensor_mul(tmp2, conv_b, s)
                nc.vector.tensor_add(btot, btot, tmp2)   # + s*conv_b
            return s, btot

        g_s, g_b = bn_fold(g_gm, g_bt, g_mn, g_vr, bg_sb, 128, 2, "g")
        neg_a = wpool.tile([CM, 1], F32, tag="neg_a")
        nc.vector.tensor_scalar(neg_a, prelu_sb, -1.0, None, ALU.mult)
        fi_s, fi_b = bn_fold(fi_gm, fi_bt, fi_mn, fi_vr, bfi_sb, 128, 2, "fi")
        m_s, m_b = bn_fold(m_gm, m_bt, m_mn, m_vr, bm_sb, CM, 1, "m")
        m_s_neg = wpool.tile([CM, 1], F32, tag="m_s_neg")
        m_b_neg = wpool.tile([CM, 1], F32, tag="m_b_neg")
        nc.vector.tensor_scalar(m_s_neg, m_s, -1.0, None, ALU.mult)
        nc.vector.tensor_scalar(m_b_neg, m_b, -1.0, None, ALU.mult)

        # Fold fi scale into WfiT weights so gat^T (n-major) needs no
        # per-channel scale: WfiT_scaled[i, o] = Wfi[o, i] * fi_s[o].
        wfi_raw = wpool.tile([128, 2, 2 * C], F32, tag="wfi_raw")
        nc.sync.dma_start(out=wfi_raw, in_=Wfi_h[:].rearrange("(a p) i -> p a i", p=128))
        for a in range(2):
            nc.scalar.activation(wfi_raw[:, a, :], wfi_raw[:, a, :], AF.Identity,
                                 bias=0.0, scale=fi_s[:, a:a + 1])
        WfiT = wpool.tile([128, 4, C], F32R, tag="wfiT")
        for a in range(2):
            for k in range(4):
                ps = pps.tile([128, 128], F32, tag="psmall")
                nc.tensor.transpose(ps, wfi_raw[:, a, k * 128:(k + 1) * 128], ident)
                nc.scalar.activation(WfiT[:, k, a * 128:(a + 1) * 128], ps, AF.Copy)
        # gat bias row (for gat^T orientation, added via rank-1 matmul):
        # fi_btot[o] = fi_s*bfi + (beta - mean*s) ... = fi_b already includes
        # conv bias term. Need it as a [1, 256] row and ones [1, 128] col.
        fi_b_row = wpool.tile([1, C], F32R, tag="fi_b_row")
        for a in range(2):
            ps = pps.tile([128, 128], F32, tag="psmall")
            nc.tensor.transpose(ps[:1, :], fi_b[:, a:a + 1], ident)
            nc.scalar.activation(fi_b_row[:, a * 128:(a + 1) * 128], ps[:1, :], AF.Copy)
        ones_row = wpool.tile([1, 128], F32R, tag="ones_row")
        nc.vector.memset(ones_row.bitcast(F32), 1.0)
        ones49 = wpool.tile([NZ, 1], F32R, tag="ones49")
        nc.vector.memset(ones49.bitcast(F32), 1.0)
        ident_bf = wpool.tile([128, 128], BF16, tag="ident_bf")
        nc.vector.tensor_copy(ident_bf, ident)
        # HAM warm-up: ~5us of dependency-free matmuls while weight DMAs
        # land, so the PE clock-gate opens (K=8/8) before real work starts
        ps_w = pps.tile([128, 128], F32, tag="psmall", name="ps_warm")
        for _ in range(48):
            nc.tensor.matmul(ps_w, ident_bf, ident_bf, start=True, stop=True)

        # 1/j constants for the sparsemax threshold, j = 1..16
        rj = wpool.tile([128, 1, 16], F32, tag="rj")
        for j in range(16):
            nc.vector.memset(rj[:, :, j:j + 1], 1.0 / (j + 1))

        # ---------------- software-pipelined per-batch stages ----------------
        # Emission order interleaves batch b's front matmuls with batch
        # b-1's sparsemax (DVE/ACT) + out-matmul so the PE never idles
        # long enough for HAM to re-throttle.
        st = [dict() for _ in range(BL)]

        def emit_load(b):
            s = st[b]
            s['xf_sb'] = dbl.tile([128, 2, NX], F32, tag="xf", name="xf_sb")
            nc.sync.dma_start(
                out=s['xf_sb'],
                in_=xf_h[b].rearrange("(a p) h w -> p a (h w)", p=128))
            s['zf_sb'] = dbl.tile([128, 2, NZ], F32, tag="zf", name="zf_sb")
            nc.sync.dma_start(
                out=s['zf_sb'],
                in_=zf_h[b].rearrange("(a p) h w -> p a (h w)", p=128))
            # round inputs to f32r right away: these gate every front matmul,
            # so they must precede the previous batch's max8s in the DVE queue
            xf_r = apool.tile([128, 2, NX + 1], F32R, tag="xf_r", name="xf_r")
            nc.vector.tensor_copy(xf_r[:, :, :NX], s['xf_sb'])
            nc.vector.memset(xf_r[:, :, NX:].bitcast(F32), 0.0)
            s['xf_r'] = xf_r
            zf_r = apool.tile([128, 2, NZ + 1], F32R, tag="zf_r", name="zf_r")
            nc.vector.tensor_copy(zf_r[:, :, :NZ], s['zf_sb'])
            nc.vector.memset(zf_r[:, :, NZ:].bitcast(F32), 0.0)
            s['zf_r'] = zf_r

        def emit_front_a(b):
            s = st[b]
            xf_r, zf_r = s['xf_r'], s['zf_r']

            # -- t path feeds sim: do it first --
            xf_t = apool.tile([128, 2, NX + 1], F32, tag="xf_t")
            nc.vector.memset(xf_t[:, :, NX:], 0.0)
            for a in range(2):
                ps_t = pp.tile([128, 1024], F32, tag="mm961")
                for f0, fw in FSPLITS:
                    sl = slice(f0, f0 + fw)
                    for k in range(2):
                        nc.tensor.matmul(
                            ps_t[:, sl], WqT[:, k, a * 128:(a + 1) * 128],
                            xf_r[:, k, sl],
                            start=(k == 0), stop=(k == 1))
                nc.scalar.activation(xf_t[:, a, :NX], ps_t[:, :NX], AF.Identity,
                                     bias=bq_sb[:, a:a + 1])
            zf_t = apool.tile([128, 2, NZ], F32, tag="zf_t")
            for a in range(2):
                ps_t = pps.tile([128, NZ + 1], F32, tag="psmall")
                for k in range(2):
                    nc.tensor.matmul(ps_t, WsT[:, k, a * 128:(a + 1) * 128],
                                     zf_r[:, k, :], start=(k == 0), stop=(k == 1))
                nc.scalar.activation(zf_t[:, a, :], ps_t[:, :NZ], AF.Identity,
                                     bias=bs_sb[:, a:a + 1])

            # simT = zf_t^T xf_t [49, 961] (fp32)
            ps_sT = pp.tile([NZ, 1024], F32, tag="mm961")
            for f0, fw in FSPLITS:
                sl = slice(f0, f0 + fw)
                for k in range(2):
                    nc.tensor.matmul(ps_sT[:, sl], zf_t[:, k, :],
                                     xf_t[:, k, sl], start=(k == 0), stop=(k == 1))
            expT = apool.tile([NZ, NX + 1], F32R, tag="expT")
            nc.vector.memset(expT[:, NX:].bitcast(F32), 0.0)
            nc.scalar.activation(expT[:, :NX], ps_sT[:, :NX], AF.Exp,
                                 bias=EXP_SHIFT)
            # unnormalized emb = zf_g @ expT can start immediately; the
            # softmax denominator commutes through this linear map and is
            # applied on the PSUM->SBUF copy below.
            ps_den = pp.tile([1, 1024], F32, tag="mm961")
            for f0, fw in FSPLITS:
                sl = slice(f0, f0 + fw)
                nc.tensor.matmul(ps_den[:, sl], ones49, expT[:, sl],
                                 start=True, stop=True)
            # rden = 1/den via exp(-ln(den)) on ACT (rden ~O(1): f32r-safe)
            lnden = apool.tile([1, NX], F32, tag="lnden")
            nc.scalar.activation(lnden, ps_den[:, :NX], AF.Ln)
            rden = apool.tile([1, NX + 1], F32R, tag="rden_row")
            nc.vector.memset(rden.bitcast(F32), 0.0)
            nc.scalar.activation(rden[:, :NX], lnden, AF.Exp, scale=-1.0)

            # -- g path fills the PE while ln/exp cook --
            xf_g = apool.tile([128, 2, NX + 1], F32R, tag="xf_g")
            nc.vector.memset(xf_g[:, :, NX:].bitcast(F32), 0.0)
            for a in range(2):
                ps_g = pp.tile([128, 1024], F32, tag="mm961")
                for f0, fw in FSPLITS:
                    sl = slice(f0, f0 + fw)
                    for k in range(2):
                        nc.tensor.matmul(
                            ps_g[:, sl], WgT[:, k, a * 128:(a + 1) * 128],
                            xf_r[:, k, sl],
                            start=(k == 0), stop=(k == 1))
                nc.scalar.activation(xf_g[:, a, :NX], ps_g[:, :NX], AF.Relu,
                                     bias=g_b[:, a:a + 1], scale=g_s[:, a:a + 1])

            # broadcast rden across partitions via rank-1 f32r matmul
            ps_bc = pp.tile([128, 1024], F32, tag="mm961")
            for f0, fw in FSPLITS:
                sl = slice(f0, f0 + fw)
                nc.tensor.matmul(ps_bc[:, sl], ones_row, rden[:, sl],
                                 start=True, stop=True)

            zf_g = apool.tile([128, 2, NZ], F32, tag="zf_g")
            for a in range(2):
                ps_g = pps.tile([128, NZ + 1], F32, tag="psmall")
                for k in range(2):
                    nc.tensor.matmul(ps_g, WgT[:, k, a * 128:(a + 1) * 128],
                                     zf_r[:, k, :], start=(k == 0), stop=(k == 1))
                nc.scalar.activation(zf_g[:, a, :], ps_g[:, :NZ], AF.Relu,
                                     bias=g_b[:, a:a + 1], scale=g_s[:, a:a + 1])
            # zf_g^T [49, 256] for emb matmul
            zf_gT = apool.tile([NZ, C], F32R, tag="zf_gT")
            for k in range(2):
                ps = pps.tile([128, 128], F32, tag="psmall")
                nc.tensor.transpose(ps[:NZ, :], zf_g[:, k, :], ident)
                nc.scalar.activation(zf_gT[:, k * 128:(k + 1) * 128], ps[:NZ, :],
                                     AF.Copy)


            rdbc = apool.tile([128, NX], F32, tag="rdbc")
            nc.scalar.activation(rdbc, ps_bc[:, :NX], AF.Copy)

            # emb = (zf_g @ expT) * rden  [256, 961]
            emb = apool.tile([128, 2, NX + 1], F32R, tag="emb")
            nc.vector.memset(emb[:, :, NX:].bitcast(F32), 0.0)
            for a in range(2):
                ps = pp.tile([128, 1024], F32, tag="mm961")
                for f0, fw in FSPLITS:
                    sl = slice(f0, f0 + fw)
                    nc.tensor.matmul(ps[:, sl],
                                     zf_gT[:, a * 128:(a + 1) * 128],
                                     expT[:, sl], start=True, stop=True)
                nc.vector.tensor_mul(emb[:, a, :NX], ps[:, :NX], rdbc)

            # gat (c-major, padded) -- double-buffered across batches
            gat_pad = dbl.tile([128, 2, PW, PWW], F32R, tag="gat_pad")
            s['gat_pad'] = gat_pad
            for a in range(2):
                nc.vector.memset(gat_pad[:, a, 0, :].bitcast(F32), 0.0)
                nc.vector.memset(gat_pad[:, a, PW - 1, :].bitcast(F32), 0.0)
                nc.vector.memset(gat_pad[:, a, 1:PW - 1, 0:1].bitcast(F32), 0.0)
                nc.vector.memset(gat_pad[:, a, 1:PW - 1, 1 + WX:PWW].bitcast(F32),
                                 0.0)
                ps = pp.tile([128, 1024], F32, tag="mm961")
                for f0, fw in FSPLITS:
                    sl = slice(f0, f0 + fw)
                    for k in range(4):
                        rhs = emb[:, k, sl] if k < 2 else xf_g[:, k - 2, sl]
                        nc.tensor.matmul(ps[:, sl],
                                         WfiT[:, k, a * 128:(a + 1) * 128],
                                         rhs, start=(k == 0), stop=(k == 3))
                nc.scalar.activation(
                    gat_pad[:, a, 1:1 + HX, 1:1 + WX],
                    ps[:, :NX].rearrange("p (h w) -> p h w", w=WX),
                    AF.Relu, bias=fi_b[:, a:a + 1])

            # contiguous bf16 copy of gat for transposing (split DVE/ACT)
            gat_sb = apool.tile([128, 2, NX], BF16, tag="gat_sb")
            nc.vector.tensor_copy(
                gat_sb[:, 0, :].rearrange("p (h w) -> p h w", w=WX),
                gat_pad[:, 0, 1:1 + HX, 1:1 + WX].bitcast(F32))
            nc.vector.tensor_copy(
                gat_sb[:, 1, :].rearrange("p (h w) -> p h w", w=WX),
                gat_pad[:, 1, 1:1 + HX, 1:1 + WX].bitcast(F32))
            # gat^T (n-major, bf16) via PE transposes (bf16: FWL + 1 cyc/col)
            gatT = dbl.tile([128, 8, C], BF16, tag="gatT")
            s['gatT'] = gatT
            for ci, (n0, cs) in enumerate(NCH):
                for a in range(2):
                    ps = pps.tile([128, 128], BF16, tag="psmall")
                    nc.tensor.transpose(ps[:cs, :], gat_sb[:, a, n0:n0 + cs],
                                        ident_bf)
                    nc.vector.tensor_copy(gatT[:cs, ci, a * 128:(a + 1) * 128],
                                          ps[:cs, :])

        def emit_conv(b):
            s = st[b]
            gat_pad = s['gat_pad']
            # e = prelu(bn(conv3x3(gat))) -> ef [64, 961(+1 pad)]
            # full 128 partitions: rows 64..127 get a DMA replica for S packing
            ef = apool.tile([2 * CM, NX + 1], F32R, tag="ef")
            nc.vector.memset(ef[:, NX:].bitcast(F32), 0.0)
            ps_e = pp.tile([CM, 1024], F32, tag="mm961")
            for r0, nr, p0 in CSPLITS:
                first = True
                for t in range(9):
                    dy, dx = t // 3, t % 3
                    for k in range(2):
                        nc.tensor.matmul(
                            ps_e[:, p0:p0 + nr * 32], WmT[:, k, t, :],
                            gat_pad[:, k, dy + r0:dy + r0 + nr, dx:dx + 32],
                            start=first, stop=(t == 8 and k == 1))
                        first = False
            # prelu(y) = relu(y) - a*relu(-y), y = m_s*conv + m_b
            efn = apool.tile([CM, NX], F32, tag="efn")
            for r0, nr, p0 in CSPLITS:
                pv = ps_e[:, p0:p0 + nr * 32].rearrange(
                    "p (r w) -> p r w", w=32)[:, :, :WX]
                ov = slice(r0 * WX, (r0 + nr) * WX)
                nc.scalar.activation(
                    ef[:CM, ov].rearrange("p (r w) -> p r w", w=WX), pv,
                    AF.Relu, bias=m_b, scale=m_s)
                nc.scalar.activation(
                    efn[:, ov].rearrange("p (r w) -> p r w", w=WX), pv,
                    AF.Relu, bias=m_b_neg, scale=m_s_neg)
                nc.vector.scalar_tensor_tensor(
                    ef[:CM, ov], efn[:, ov], neg_a[:, 0:1],
                    ef[:CM, ov].bitcast(F32), op0=ALU.mult, op1=ALU.add)
                # replicate this split to partitions 64..127 immediately
                nc.sync.dma_start(out=ef[CM:2 * CM, ov], in_=ef[:CM, ov])
            nc.sync.dma_start(out=ef[CM:2 * CM, NX:], in_=ef[:CM, NX:])

            s['ef'] = ef

        def emit_S(b, early_max8=False):
            s = st[b]
            ef = s['ef']
            # S = ef^T ef  [961, 961]; chunk pairs run concurrently in PE
            # row-groups 0-63 / 64-127 (K=64 each)
            S_sb = spool.tile([128, 8, NX], F32, tag="S")
            s['S_sb'] = S_sb
            if early_max8:
                v16 = vpool.tile([128, 8, 16], F32, tag="v16", name="v16")
                nc.vector.memset(v16, 0.0)
                s['v16'] = v16
            for p in range(4):
                c0, c1 = 2 * p, 2 * p + 1
                n00, cs0 = NCH[c0]
                n01, cs1 = NCH[c1]
                psA = pp.tile([128, 1024], F32, tag="mm961")
                psB = pp.tile([128, 1024], F32, tag="mm961")
                for f0, fw in FSPLITS:
                    sl = slice(f0, f0 + fw)
                    nc.tensor.matmul(psA[:cs0, sl], ef[:CM, n00:n00 + cs0],
                                     ef[:CM, sl], start=True, stop=True)
                    nc.tensor.matmul(psB[:cs1, sl],
                                     ef[CM:2 * CM, n01:n01 + cs1],
                                     ef[CM:2 * CM, sl], start=True, stop=True)
                nc.vector.tensor_copy(S_sb[:cs0, c0, :], psA[:cs0, :NX])
                nc.scalar.activation(S_sb[:cs1, c1, :], psB[:cs1, :NX], AF.Copy)
                if early_max8:
                    for cc, css in ((c0, cs0), (c1, cs1)):
                        nc.vector.max(out=s['v16'][:css, cc, 0:8],
                                      in_=S_sb[:css, cc, 0:NX:2])
                        nc.vector.max(out=s['v16'][:css, cc, 8:16][:, ::-1],
                                      in_=S_sb[:css, cc, 1:NX:2])

        def emit_topk(b):
            s = st[b]
            S_sb = s['S_sb']
            if 'v16' in s:
                v16 = s['v16']
            else:
                v16 = vpool.tile([128, 8, 16], F32, tag="v16")
                nc.vector.memset(v16, 0.0)
                for ci, (n0, cs) in enumerate(NCH):
                    nc.vector.max(out=v16[:cs, ci, 0:8],
                                  in_=S_sb[:cs, ci, 0:NX:2])
                    nc.vector.max(out=v16[:cs, ci, 8:16][:, ::-1],
                                  in_=S_sb[:cs, ci, 1:NX:2])
            mA = vpool.tile([128, 8, 16], F32, tag="mA")
            mB = vpool.tile([128, 8, 16], F32, tag="mB")
            # bitonic merge (descending) of [desc8 ; asc8]
            nc.vector.tensor_tensor(mA[:, :, 0:8], v16[:, :, 0:8],
                                    v16[:, :, 8:16], ALU.max)
            nc.vector.tensor_tensor(mA[:, :, 8:16], v16[:, :, 0:8],
                                    v16[:, :, 8:16], ALU.min)
            for (mm0, mm1, g, x) in ((mA, mB, 2, 8), (mB, mA, 4, 4),
                                     (mA, mB, 8, 2)):
                s4 = mm0.rearrange("p c (g x) -> p c g x", g=g)
                d4 = mm1.rearrange("p c (g x) -> p c g x", g=g)
                h = x // 2
                nc.vector.tensor_tensor(d4[:, :, :, 0:h], s4[:, :, :, 0:h],
                                        s4[:, :, :, h:x], ALU.max)
                nc.vector.tensor_tensor(d4[:, :, :, h:x], s4[:, :, :, 0:h],
                                        s4[:, :, :, h:x], ALU.min)
            # mB sorted descending; cumsum (Hillis-Steele, ping-pong)
            for (mm0, mm1, sh) in ((mB, mA, 1), (mA, v16, 2), (v16, mA, 4),
                                   (mA, v16, 8)):
                nc.vector.tensor_tensor(mm1[:, :, sh:16], mm0[:, :, sh:16],
                                        mm0[:, :, 0:16 - sh], ALU.add)
                nc.vector.tensor_copy(mm1[:, :, 0:sh], mm0[:, :, 0:sh])
            # t_j = (cumsum_j - 1) / j ; tau = max_j t_j
            nc.vector.scalar_tensor_tensor(mB, v16, 1.0,
                                           rj.to_broadcast([128, 8, 16]),
                                           op0=ALU.subtract, op1=ALU.mult)
            tau = vpool.tile([128, 8], F32, tag="tau")
            nc.vector.tensor_reduce(tau, mB, mybir.AxisListType.X, ALU.max)
            ntau = vpool.tile([128, 8], F32, tag="ntau")
            nc.vector.tensor_scalar(ntau, tau, -1.0, None, ALU.mult)
            s['tau'] = tau
            s['ntau'] = ntau

        def emit_P(b):
            # P = relu(S - tau) in bf16, split DVE/ACT; emitted late so the
            # ACT chunks don't head-of-line-block the next batch's copies
            s = st[b]
            S_sb, tau, ntau = s['S_sb'], s['tau'], s['ntau']
            P_sb = spool.tile([128, 8, NX], BF16, tag="P")
            s['P_sb'] = P_sb
            for ci, (n0, cs) in enumerate(NCH):
                if ci % 2 == 0:
                    nc.vector.tensor_scalar(P_sb[:cs, ci, :], S_sb[:cs, ci, :],
                                            tau[:cs, ci:ci + 1], 0.0,
                                            ALU.subtract, ALU.max)
                else:
                    nc.scalar.activation(P_sb[:cs, ci, :], S_sb[:cs, ci, :],
                                         AF.Relu, bias=ntau[:cs, ci:ci + 1])

        def emit_out(b):
            s = st[b]
            gat_pad, gatT, P_sb = s['gat_pad'], s['gatT'], s['P_sb']
            out_sb = dbl.tile([128, 2, NX], F32, tag="out_sb")
            for a in range(2):
                ps = pp.tile([128, 1024], F32, tag="mm961")
                for f0, fw in OSPLITS:
                    sl = slice(f0, f0 + fw)
                    for ci, (n0, cs) in enumerate(NCH):
                        nc.tensor.matmul(ps[:, sl],
                                         gatT[:cs, ci, a * 128:(a + 1) * 128],
                                         P_sb[:cs, ci, sl],
                                         start=(ci == 0), stop=(ci == 7))
                nc.vector.tensor_add(
                    out_sb[:, a, :].rearrange("p (h w) -> p h w", w=WX),
                    ps[:, :NX].rearrange("p (h w) -> p h w", w=WX),
                    gat_pad[:, a, 1:1 + HX, 1:1 + WX].bitcast(F32))
            nc.sync.dma_start(
                out=out_h[b].rearrange("(a p) h w -> p a (h w)", p=128),
                in_=out_sb)

        emit_load(0)
        emit_front_a(0)
        emit_conv(0)
        emit_S(0)
        for b in range(1, BL):
            emit_load(b)
            emit_topk(b - 1)
            emit_front_a(b)
            emit_conv(b)
            emit_P(b - 1)
            emit_out(b - 1)
            emit_S(b, early_max8=(b == BL - 1))
        emit_topk(BL - 1)
        emit_P(BL - 1)
        emit_out(BL - 1)

    nc.compile()
    return nc


_CACHED = None


def _get_nc():
    global _CACHED
    if _CACHED is None:
        _CACHED = build_bass()
    return _CACHED


def kernel(**inputs):
    nc = _get_nc()
    full = {k: np.ascontiguousarray(np.asarray(v, dtype=np.float32))
            for k, v in inputs.items()}
    full['prelu_a'] = full['prelu_a'].reshape(1)
    in_maps = []
    for c in range(NCORES):
        m = dict(full)
        m['zf'] = full['zf'][c * BL:(c + 1) * BL]
        m['xf'] = full['xf'][c * BL:(c + 1) * BL]
        in_maps.append(m)
    res = run_bass_kernel_spmd(nc, in_maps, core_ids=list(range(NCORES)))
    out = np.concatenate([r['out'] for r in res.results], axis=0)
    return out.astype(np.float32)


if __name__ == "__main__":
    # smoke-build
    nc = build_bass()
    print("built ok:",
          sum(len(b.instructions) for f in nc.m.functions for b in f.blocks),
          "instructions")
